# revision 1
# baseline (speedup 1.0000x reference)
"""Trainium2 Bass kernel for the additive-attention glimpse module.

Math (per batch b):
    qp  = query @ Wq.T + bq                       # [E]
    cp  = context @ Wc.T + bc                     # [N, E]
    comb = tanh(qp + cp)                          # [N, E]
    attn = comb @ Wo.T (+ bo, softmax-invariant)  # [N, G]
    w    = softmax(attn, axis=N)                  # [N, G]
    out  = (w.T @ context).reshape(G*Cd)          # [G*Cd]

Shapes: B=256, N=196, Cd=2048, Qd=E=1024, G=8.

Strategy: data-parallel over B across 8 cores (32 batches each). On each
core everything is kept feature-on-partition ("transposed" layout) so the
dominant matmul (context @ Wc.T, ~26 GFLOP/core) runs as
cp.T[e, r] = WcT.T @ ctx.T with bf16 operands at 1 cycle/row. ctx.T tiles
come from HBM via the DMA xbar transpose (bf16-only path), so the
TensorEngine spends no cycles transposing the big tensor. The natural
[n, c] layout (needed by the glimpse matmul, which contracts over n) is a
second, plain DMA of the same bf16 array.
"""

import numpy as np
import ml_dtypes

BF16 = ml_dtypes.bfloat16

B_FULL = 256
N_CTX = 196
CD = 2048
QD = 1024
E = 1024
G = 8
N_CORES = 8
B_LOC = B_FULL // N_CORES  # 32

SLAB_B = 4          # batches per transpose-DMA slab (rows 4*196=784, /16 ok)
CHUNK_B = 2         # batches per compute chunk (rows 392 <= 512 psum bank)
CHUNK_R = CHUNK_B * N_CTX  # 392


def build_nc(b_loc=B_LOC, reps=1, rep_scales=None, probe=None,
             attn_split=False, plain_xt_dma=False, ldw_pair=False,
             row_split=False, epair=False):
    """Build the single-core Bass/Tile graph (SPMD: same graph on all cores).

    reps>1 repeats the whole computation (same inputs -> same outputs)
    inside one NEFF; used only for wall-clock HW timing, since per-execute
    RPC overhead in this container is ~100ms.
    """
    import concourse.mybir as mybir
    import concourse.tile as tile
    from concourse import bacc
    from concourse.masks import make_identity

    f32 = mybir.dt.float32
    bf16 = mybir.dt.bfloat16
    Act = mybir.ActivationFunctionType
    Alu = mybir.AluOpType

    attn_split = attn_split or ldw_pair
    assert b_loc % SLAB_B == 0
    n_slab = b_loc // SLAB_B
    R = b_loc * N_CTX

    nc = bacc.Bacc("TRN2", target_bir_lowering=False, debug=False,
                   num_devices=N_CORES)

    ctx = nc.dram_tensor("ctx", [R, CD], bf16, kind="ExternalInput").ap()
    qT = nc.dram_tensor("qT", [QD, b_loc], f32, kind="ExternalInput").ap()
    WqT = nc.dram_tensor("WqT", [QD, E], f32, kind="ExternalInput").ap()
    WcT = nc.dram_tensor("WcT", [CD, E], bf16, kind="ExternalInput").ap()
    WoT = nc.dram_tensor("WoT", [E, G], bf16, kind="ExternalInput").ap()
    bqc = nc.dram_tensor("bqc", [128, E // 128], f32, kind="ExternalInput").ap()
    out = nc.dram_tensor("out", [b_loc, G * CD], f32, kind="ExternalOutput").ap()

    NE = E // 128    # 8 e-tiles
    NCC = CD // 128  # 16 c-tiles
    NQ = QD // 128   # 8 q-tiles

    with tile.TileContext(nc) as tc:
        with (
            tc.tile_pool(name="const", bufs=1) as const_pool,
            tc.tile_pool(name="xt", bufs=2) as xt_pool,
            tc.tile_pool(name="nat", bufs=2) as nat_pool,
            tc.tile_pool(name="comb", bufs=3) as comb_pool,
            tc.tile_pool(name="sm", bufs=8) as sm_pool,
            tc.tile_pool(name="wl", bufs=8) as wl_pool,
            tc.tile_pool(name="outb", bufs=2) as outb_pool,
            tc.tile_pool(name="tmp", bufs=3) as tmp_pool,
            tc.tile_pool(name="pcp",
                         bufs=(4 if (ldw_pair or row_split or epair) else 2),
                         space="PSUM") as pc_pool,
            tc.tile_pool(name="pat", bufs=2, space="PSUM") as pa_pool,
            tc.tile_pool(name="pgl",
                         bufs=(2 if (ldw_pair or row_split or epair) else 4),
                         space="PSUM") as pg_pool,
        ):
            # ---- persistent constants ----
            wc_sb = const_pool.tile([128, NCC, E], bf16)
            nc.sync.dma_start(wc_sb[:], WcT.rearrange("(k p) e -> p k e", p=128))
            wo_sb = const_pool.tile([128, NE, G], bf16)
            nc.sync.dma_start(wo_sb[:], WoT.rearrange("(k p) g -> p k g", p=128))
            bqc_sb = const_pool.tile([128, NE], f32)
            nc.sync.dma_start(bqc_sb[:], bqc[:])
            ident = const_pool.tile([128, 128], bf16)
            make_identity(nc, ident[:])

            qpb_sb = const_pool.tile([128, NE, b_loc], f32, tag="qpb")

            def one_pass(out_scale=1.0):
                # ---- qp = Wq @ query.T (+bq+bc), kept as [e, b] ----
                qt_sb = wq_pool.tile([128, NQ, b_loc], f32, tag="qt")
                nc.sync.dma_start(qt_sb[:], qT.rearrange("(k p) b -> p k b", p=128))
                for e in range(NE):
                    wq_sb = wq_pool.tile([128, NQ, 128], f32, tag="wqe")
                    nc.sync.dma_start(
                        wq_sb[:],
                        WqT[:, e * 128:(e + 1) * 128].rearrange(
                            "(k p) m -> p k m", p=128))
                    pq = pc_pool.tile([128, b_loc], f32, tag="pcp")
                    for k in range(NQ):
                        nc.tensor.matmul(
                            pq[:], wq_sb[:, k, :],
                            qt_sb[:, k, :], start=(k == 0), stop=(k == NQ - 1),
                        )
                    nc.vector.tensor_scalar_add(
                        qpb_sb[:, e, :], pq[:], bqc_sb[:, e:e + 1])

                # ---- main loop over 4-batch slabs ----
                for s in range(n_slab):
                    r0 = s * SLAB_B * N_CTX
                    xt = xt_pool.tile([128, NCC, SLAB_B * N_CTX], bf16, tag="xt")
                    for c in range(NCC):
                        if plain_xt_dma:
                            nc.sync.dma_start(
                                xt[:, c, :],
                                ctx[r0:r0 + 128, c * SLAB_B * N_CTX // 128:
                                    c * SLAB_B * N_CTX // 128 + SLAB_B * N_CTX])
                        else:
                            nc.sync.dma_start_transpose(
                                xt[:, c, :], ctx[r0:r0 + SLAB_B * N_CTX,
                                                 c * 128:(c + 1) * 128])

                    if probe is None:
                        nat_a = nat_pool.tile([128, SLAB_B, CD], bf16, tag="nat_a")
                        nat_b = nat_pool.tile([68, SLAB_B, CD], bf16, tag="nat_b")
                        for j in range(SLAB_B):
                            rb = r0 + j * N_CTX
                            nc.sync.dma_start(nat_a[:, j, :], ctx[rb:rb + 128, :])
                            nc.sync.dma_start(nat_b[:, j, :],
                                              ctx[rb + 128:rb + N_CTX, :])

                    if ldw_pair:
                        NH = SLAB_B // CHUNK_B
                        combs = [comb_pool.tile([128, NE, CHUNK_R], bf16,
                                                tag="comb", name=f"comb{_h}")
                                 for _h in range(NH)]
                        for e in range(NE):
                            pcs = [pc_pool.tile([128, CHUNK_R], f32,
                                                tag="pcp", name=f"pc{_h}")
                                   for _h in range(NH)]
                            for c in range(NCC):
                                for h in range(NH):
                                    nc.tensor.matmul(
                                        pcs[h][:],
                                        wc_sb[:, c, e * 128:(e + 1) * 128],
                                        xt[:, c, h * CHUNK_R:(h + 1) * CHUNK_R],
                                        start=(c == 0), stop=(c == NCC - 1),
                                    )
                            for h in range(NH):
                                b0 = s * SLAB_B + h * CHUNK_B
                                for j in range(CHUNK_B):
                                    nc.scalar.activation(
                                        combs[h][:, e, j * N_CTX:(j + 1) * N_CTX],
                                        pcs[h][:, j * N_CTX:(j + 1) * N_CTX],
                                        Act.Tanh,
                                        bias=qpb_sb[:, e, b0 + j:b0 + j + 1],
                                    )
                        pa_tiles = []
                        for h in range(NH):
                            pa = pa_pool.tile([128, N_CTX], f32, tag="pat")
                            for e in range(NE):
                                for jj in range(CHUNK_B):
                                    nc.tensor.matmul(
                                        pa[32 * jj:32 * jj + G, :],
                                        wo_sb[:, e, :],
                                        combs[h][:, e,
                                                 jj * N_CTX:(jj + 1) * N_CTX],
                                        start=(e == 0), stop=(e == NE - 1),
                                        tile_position=(0, 32 * jj),
                                        skip_group_check=True,
                                    )
                            asb = sm_pool.tile([128, N_CTX], f32, tag="asb")
                            for jj in range(CHUNK_B):
                                nc.scalar.activation(
                                    asb[32 * jj:32 * jj + G, :],
                                    pa[32 * jj:32 * jj + G, :], Act.Copy)
                            pa_tiles.append(asb)
                    else:
                      pa_tiles = []
                    if not ldw_pair:
                      for h in range(SLAB_B // CHUNK_B):
                          if probe in ("mm_cycle", "mm_fixed"):
                              for e in range(NE):
                                  pc = pc_pool.tile([128, CHUNK_R], f32, tag="pcp")
                                  for c in range(NCC):
                                      w_ap = (wc_sb[:, 0, 0:128]
                                              if probe == "mm_fixed" else
                                              wc_sb[:, c, e * 128:(e + 1) * 128])
                                      nc.tensor.matmul(
                                          pc[:], w_ap,
                                          xt[:, c, h * CHUNK_R:(h + 1) * CHUNK_R],
                                          start=(c == 0), stop=(c == NCC - 1),
                                      )
                              continue
                          b0 = s * SLAB_B + h * CHUNK_B
                          rc = h * CHUNK_R  # offset inside the slab

                          # cp.T tiles + fused bias/tanh -> comb.T (bf16)
                          comb = comb_pool.tile([128, NE, CHUNK_R], bf16, tag="comb")
                          for e in range(NE):
                              if epair:
                                  if e % 2 == 1:
                                      continue  # handled with its even partner
                                  pc0 = pc_pool.tile([128, CHUNK_R], f32,
                                                     tag="pcp", name="pe0")
                                  pc1 = pc_pool.tile([128, CHUNK_R], f32,
                                                     tag="pcp", name="pe1")
                                  for c in range(NCC):
                                      nc.tensor.matmul(
                                          pc0[:],
                                          wc_sb[:, c, e * 128:(e + 1) * 128],
                                          xt[:, c, rc:rc + CHUNK_R],
                                          start=(c == 0), stop=(c == NCC - 1),
                                      )
                                      nc.tensor.matmul(
                                          pc1[:],
                                          wc_sb[:, c,
                                                (e + 1) * 128:(e + 2) * 128],
                                          xt[:, c, rc:rc + CHUNK_R],
                                          start=(c == 0), stop=(c == NCC - 1),
                                      )
                                  for i, pc in enumerate((pc0, pc1)):
                                      for j in range(CHUNK_B):
                                          nc.scalar.activation(
                                              comb[:, e + i,
                                                   j * N_CTX:(j + 1) * N_CTX],
                                              pc[:, j * N_CTX:(j + 1) * N_CTX],
                                              Act.Tanh,
                                              bias=qpb_sb[:, e + i,
                                                          b0 + j:b0 + j + 1],
                                          )
                                  continue
                              if row_split:
                                  # two K=64 row-tiles run concurrently in the
                                  # PE array; separate PSUM banks, DVE merge
                                  pca = pc_pool.tile([128, CHUNK_R], f32,
                                                     tag="pcp", name="pca")
                                  pcb = pc_pool.tile([128, CHUNK_R], f32,
                                                     tag="pcp", name="pcb")
                                  es = slice(e * 128, (e + 1) * 128)
                                  for c in range(NCC):
                                      nc.tensor.matmul(
                                          pca[:], wc_sb[0:64, c, es],
                                          xt[0:64, c, rc:rc + CHUNK_R],
                                          start=(c == 0), stop=(c == NCC - 1),
                                          tile_position=(0, 0),
                                      )
                                      nc.tensor.matmul(
                                          pcb[:], wc_sb[64:128, c, es],
                                          xt[64:128, c, rc:rc + CHUNK_R],
                                          start=(c == 0), stop=(c == NCC - 1),
                                          tile_position=(64, 0),
                                      )
                                  tmpa = tmp_pool.tile([128, CHUNK_R], f32,
                                                       tag="tmpa", name="tmpa")
                                  nc.scalar.activation(tmpa[:], pca[:], Act.Copy)
                                  pc = tmp_pool.tile([128, CHUNK_R], f32,
                                                     tag="tmp", name="tmp")
                                  nc.vector.tensor_add(pc[:], tmpa[:], pcb[:])
                              else:
                                  pc = pc_pool.tile([128, CHUNK_R], f32, tag="pcp")
                                  for c in range(NCC):
                                      nc.tensor.matmul(
                                          pc[:], wc_sb[:, c, e * 128:(e + 1) * 128],
                                          xt[:, c, rc:rc + CHUNK_R],
                                          start=(c == 0), stop=(c == NCC - 1),
                                      )
                              for j in range(CHUNK_B):
                                  nc.scalar.activation(
                                      comb[:, e, j * N_CTX:(j + 1) * N_CTX],
                                      pc[:, j * N_CTX:(j + 1) * N_CTX],
                                      Act.Tanh,
                                      bias=qpb_sb[:, e, b0 + j:b0 + j + 1],
                                  )

                          if probe is not None:
                              continue
                          # attn.T = WoT.T @ comb.T  -> [G, 392]
                          if attn_split:
                              # batch jj in PE column group jj (partitions 32*jj)
                              pa = pa_pool.tile([128, N_CTX], f32, tag="pat")
                              for e in range(NE):
                                  for jj in range(CHUNK_B):
                                      nc.tensor.matmul(
                                          pa[32 * jj:32 * jj + G, :],
                                          wo_sb[:, e, :],
                                          comb[:, e, jj * N_CTX:(jj + 1) * N_CTX],
                                          start=(e == 0), stop=(e == NE - 1),
                                          tile_position=(0, 32 * jj),
                                          skip_group_check=True,
                                      )
                          else:
                              pa = pa_pool.tile([G, CHUNK_R], f32, tag="pat")
                              for e in range(NE):
                                  nc.tensor.matmul(
                                      pa[:], wo_sb[:, e, :], comb[:, e, :],
                                      start=(e == 0), stop=(e == NE - 1),
                                  )
                          pa_tiles.append(pa)

                    if probe is not None:
                        continue
                    # per-batch softmax + weight transpose for the whole slab
                    wls, rss = [], []
                    for j in range(SLAB_B):
                        pa = pa_tiles[j // CHUNK_B]
                        jj = j % CHUNK_B
                        if attn_split:
                            seg = pa[32 * jj:32 * jj + G, :]
                        else:
                            seg = pa[:, jj * N_CTX:(jj + 1) * N_CTX]
                        nmx = sm_pool.tile([G, 1], f32, tag="nmx")
                        nc.vector.tensor_reduce(
                            nmx[:], seg, axis=mybir.AxisListType.X,
                            op=Alu.max, negate=True)
                        wex = sm_pool.tile([G, N_CTX], bf16, tag="wex")
                        ssum = sm_pool.tile([G, 1], f32, tag="ssum")
                        nc.scalar.activation(
                            wex[:], seg, Act.Exp, bias=nmx[:],
                            accum_out=ssum[:])
                        rs = sm_pool.tile([G, 1], f32, tag="rs")
                        nc.vector.reciprocal(rs[:], ssum[:])
                        if out_scale != 1.0:
                            nc.vector.tensor_scalar_mul(
                                rs[:], rs[:], float(out_scale))
                        rss.append(rs)

                        wla = wl_pool.tile([128, G], bf16, tag="wla")
                        wlb = wl_pool.tile([68, G], bf16, tag="wlb")
                        pta = pa_pool.tile([128, G], bf16, tag="pat")
                        nc.tensor.transpose(pta[:], wex[:, 0:128],
                                            ident[:G, :G])
                        nc.vector.tensor_copy(wla[:], pta[:])
                        ptb = pa_pool.tile([68, G], bf16, tag="pat")
                        nc.tensor.transpose(ptb[:], wex[:, 128:N_CTX],
                                            ident[:G, :G])
                        nc.vector.tensor_copy(wlb[:], ptb[:])
                        wls.append((wla, wlb))

                    # glimpse for 4 batches concurrently via PE column tiling:
                    # batch j occupies column group j (out partitions 32j..32j+7)
                    outb = outb_pool.tile([128, CD], f32, tag="outb")
                    for cc in range(CD // 512):
                        pg = pg_pool.tile([128, 512], f32, tag="pgl")
                        for j in range(SLAB_B):
                            nc.tensor.matmul(
                                pg[32 * j:32 * j + G, :], wls[j][0][:],
                                nat_a[:, j, cc * 512:(cc + 1) * 512],
                                start=True, stop=False,
                                tile_position=(0, 32 * j),
                                skip_group_check=True)
                        for j in range(SLAB_B):
                            nc.tensor.matmul(
                                pg[32 * j:32 * j + G, :], wls[j][1][:],
                                nat_b[:, j, cc * 512:(cc + 1) * 512],
                                start=False, stop=True,
                                tile_position=(0, 32 * j),
                                skip_group_check=True)
                        for j in range(SLAB_B):
                            dst = outb[32 * j:32 * j + G,
                                       cc * 512:(cc + 1) * 512]
                            if j % 2 == 0:
                                nc.vector.tensor_scalar_mul(
                                    dst, pg[32 * j:32 * j + G, :], rss[j][:])
                            else:
                                nc.scalar.activation(
                                    dst, pg[32 * j:32 * j + G, :],
                                    Act.Identity, bias=0.0, scale=rss[j][:])

                    for j in range(SLAB_B):
                        nc.gpsimd.dma_start(
                            out[s * SLAB_B + j, :].rearrange(
                                "(g c) -> g c", g=G),
                            outb[32 * j:32 * j + G, :])

            with tc.tile_pool(name="wq", bufs=2) as wq_pool:
                for _rep in range(reps):
                    one_pass(out_scale=rep_scales[_rep] if rep_scales else 1.0)

    nc.compile()
    return nc


_NC_CACHE = {}


def _get_nc(b_loc=B_LOC):
    if b_loc not in _NC_CACHE:
        _NC_CACHE[b_loc] = build_nc(b_loc, attn_split=True)
    return _NC_CACHE[b_loc]


def make_in_maps(context, query, Wq, bq, Wc, bc, Wo, bo, b_loc=B_LOC,
                 n_cores=N_CORES):
    """Host-side prep: dtype conversion, weight transposes, sharding."""
    context = np.asarray(context)
    query = np.asarray(query)
    Wq, bq = np.asarray(Wq), np.asarray(bq)
    Wc, bc = np.asarray(Wc), np.asarray(bc)
    Wo = np.asarray(Wo)
    ctx_bf = np.ascontiguousarray(context).astype(BF16)
    WqT = np.ascontiguousarray(Wq.T.astype(np.float32))
    WcT = np.ascontiguousarray(Wc.T).astype(BF16)
    WoT = np.ascontiguousarray(Wo.T).astype(BF16)
    bqc = np.ascontiguousarray(
        (bq + bc).astype(np.float32).reshape(E // 128, 128).T)
    in_maps = []
    for i in range(n_cores):
        b0 = i * b_loc
        in_maps.append(dict(
            ctx=ctx_bf[b0:b0 + b_loc].reshape(b_loc * N_CTX, CD),
            qT=np.ascontiguousarray(query[b0:b0 + b_loc].T.astype(np.float32)),
            WqT=WqT, WcT=WcT, WoT=WoT, bqc=bqc,
        ))
    return in_maps


def kernel(context, query, Wq, bq, Wc, bc, Wo, bo):
    from concourse.bass_utils import run_bass_kernel_spmd

    assert context.shape == (B_FULL, N_CTX, CD)
    nc = _get_nc()
    in_maps = make_in_maps(context, query, Wq, bq, Wc, bc, Wo, bo)
    res = run_bass_kernel_spmd(nc, in_maps, core_ids=list(range(N_CORES)))
    return np.concatenate([res.results[i]["out"] for i in range(N_CORES)],
                          axis=0)



# revision 2
# speedup vs baseline: 1.6422x; 1.6422x over previous
"""Hybrid fp8-DoubleRow / bf16 Trainium2 kernel for the additive-attention
glimpse module.

Math (per batch b):
    qp  = query @ Wq.T + bq                       # [E]
    cp  = context @ Wc.T + bc                     # [N, E]
    comb = tanh(qp + cp)                          # [N, E]
    attn = comb @ Wo.T (+ bo, softmax-invariant)  # [N, G]
    w    = softmax(attn, axis=N)                  # [N, G]
    out  = (w.T @ context).reshape(G*Cd)          # [G*Cd]

Shapes: B=256, N=196, Cd=2048, Qd=E=1024, G=8.  Data-parallel over B on 8
cores (32 batches each).

The dominant matmul cp.T = Wc @ ctx.T (~26 GFLOP/core) runs with the first
`2*fp8_pairs` of the 16 contraction k-tiles as fp8e4m3 DoubleRow pairs
(2 k-tiles per PE pass, 2x throughput) and the rest in bf16.  Both operand
halves are pre-transposed and pre-quantized on the host; Wc is pre-scaled
by 32 (fp8 subnormal avoidance) and the tanh activation applies the 1/32
compensation via its scale parameter.
"""

import numpy as np
import ml_dtypes

BF16 = ml_dtypes.bfloat16
FP8 = ml_dtypes.float8_e4m3  # IEEE-style e4m3, max 240 == TRN FP8_EXP4

B_FULL = 256
N_CTX = 196
CD = 2048
QD = 1024
E = 1024
G = 8
N_CORES = 8
B_LOC = B_FULL // N_CORES  # 32

SLAB_B = 4                  # batches per slab
CHUNK_B = 2                 # batches per psum chunk (392 <= 512 psum bank)
CHUNK_R = CHUNK_B * N_CTX   # 392
R_SLAB = SLAB_B * N_CTX     # 784

WC_SCALE = 32.0             # host pre-scale on Wc (both halves)

NE = E // 128    # 8 e-tiles
NCC = CD // 128  # 16 c-tiles
NQ = QD // 128   # 8 q-tiles


def build_nc(b_loc=B_LOC, fp8_pairs=6, reps=1, probe=None, swi=True):
    import concourse.mybir as mybir
    import concourse.tile as tile
    from concourse import bacc
    from concourse.masks import make_identity

    f32 = mybir.dt.float32
    bf16 = mybir.dt.bfloat16
    fp8 = mybir.dt.float8e4
    Act = mybir.ActivationFunctionType
    Alu = mybir.AluOpType
    DR = (mybir.MatmulPerfMode.DoubleRowSwInterleave if swi
          else mybir.MatmulPerfMode.DoubleRow)

    n8 = 2 * fp8_pairs       # fp8 k-tiles
    n16 = NCC - n8           # bf16 k-tiles
    assert 0 <= n8 <= NCC

    assert b_loc % SLAB_B == 0
    n_slab = b_loc // SLAB_B
    R = b_loc * N_CTX

    nc = bacc.Bacc("TRN2", target_bir_lowering=False, debug=False,
                   num_devices=N_CORES)

    ctx_nat = nc.dram_tensor("ctxn", [R, CD], bf16, kind="ExternalInput").ap()
    if n8:
        ctx8T = nc.dram_tensor("ctx8T", [n8 * 128, R], fp8,
                               kind="ExternalInput").ap()
        if swi:
            wc8T = nc.dram_tensor("wc8T", [fp8_pairs * 128, NE * 256], fp8,
                                  kind="ExternalInput").ap()
        else:
            wc8T = nc.dram_tensor("wc8T", [n8 * 128, E], fp8,
                                  kind="ExternalInput").ap()
    if n16:
        ctx16T = nc.dram_tensor("ctx16T", [n16 * 128, R], bf16,
                                kind="ExternalInput").ap()
        wc16T = nc.dram_tensor("wc16T", [n16 * 128, E], bf16,
                               kind="ExternalInput").ap()
    qT = nc.dram_tensor("qT", [QD, b_loc], bf16, kind="ExternalInput").ap()
    WqT = nc.dram_tensor("WqT", [QD, E], bf16, kind="ExternalInput").ap()
    WoT = nc.dram_tensor("WoT", [E, G], bf16, kind="ExternalInput").ap()
    bqc = nc.dram_tensor("bqc", [128, E // 128], f32, kind="ExternalInput").ap()
    out = nc.dram_tensor("out", [b_loc, G * CD], f32, kind="ExternalOutput").ap()

    with tile.TileContext(nc) as tc:
        with (
            tc.tile_pool(name="const", bufs=1) as const_pool,
            tc.tile_pool(name="xt", bufs=2) as xt_pool,
            tc.tile_pool(name="nat", bufs=2) as nat_pool,
            tc.tile_pool(name="comb", bufs=4) as comb_pool,
            tc.tile_pool(name="sm", bufs=8) as sm_pool,
            tc.tile_pool(name="wl", bufs=4) as wl_pool,
            tc.tile_pool(name="outb", bufs=2) as outb_pool,
            tc.tile_pool(name="pcp", bufs=4, space="PSUM") as pc_pool,
            tc.tile_pool(name="pat", bufs=2, space="PSUM") as pa_pool,
            tc.tile_pool(name="pgl", bufs=2, space="PSUM") as pg_pool,
        ):
            # ---- persistent constants ----
            if n8:
                if swi:
                    wc8_sb = const_pool.tile([128, fp8_pairs, NE * 256], fp8)
                    nc.sync.dma_start(
                        wc8_sb[:], wc8T.rearrange("(k p) x -> p k x", p=128))
                else:
                    wc8_sb = const_pool.tile([128, n8, E], fp8)
                    nc.sync.dma_start(
                        wc8_sb[:], wc8T.rearrange("(k p) e -> p k e", p=128))
            if n16:
                wc16_sb = const_pool.tile([128, n16, E], bf16)
                nc.sync.dma_start(
                    wc16_sb[:], wc16T.rearrange("(k p) e -> p k e", p=128))
            wo_sb = const_pool.tile([128, NE, G], bf16)
            nc.sync.dma_start(wo_sb[:], WoT.rearrange("(k p) g -> p k g", p=128))
            bqc_sb = const_pool.tile([128, NE], f32)
            nc.sync.dma_start(bqc_sb[:], bqc[:])
            ident = const_pool.tile([128, 128], bf16)
            make_identity(nc, ident[:])

            qpb_sb = const_pool.tile([128, NE, b_loc], f32, tag="qpb")

            def one_pass(out_scale=1.0):
                # ---- qp = Wq @ query.T (+bq+bc), kept as [e, b] ----
                qt_sb = wq_pool.tile([128, NQ, b_loc], bf16, tag="qt")
                nc.sync.dma_start(qt_sb[:], qT.rearrange("(k p) b -> p k b", p=128))
                for e in range(NE):
                    wq_sb = wq_pool.tile([128, NQ, 128], bf16, tag="wqe")
                    nc.sync.dma_start(
                        wq_sb[:],
                        WqT[:, e * 128:(e + 1) * 128].rearrange(
                            "(k p) m -> p k m", p=128))
                    pq = pc_pool.tile([128, b_loc], f32, tag="pcp")
                    for k in range(NQ):
                        nc.tensor.matmul(
                            pq[:], wq_sb[:, k, :],
                            qt_sb[:, k, :], start=(k == 0), stop=(k == NQ - 1),
                        )
                    nc.vector.tensor_scalar_add(
                        qpb_sb[:, e, :], pq[:], bqc_sb[:, e:e + 1])

                # ---- main loop over 4-batch slabs ----
                for s in range(n_slab):
                    r0 = s * R_SLAB
                    if n8:
                        xt8 = xt_pool.tile([128, n8, R_SLAB], fp8, tag="xt8")
                        nc.sync.dma_start(
                            xt8[:],
                            ctx8T.rearrange("(k p) r -> p k r",
                                            p=128)[:, :, r0:r0 + R_SLAB])
                    if n16:
                        xt16 = xt_pool.tile([128, n16, R_SLAB], bf16, tag="xt16")
                        nc.sync.dma_start(
                            xt16[:],
                            ctx16T.rearrange("(k p) r -> p k r",
                                             p=128)[:, :, r0:r0 + R_SLAB])

                    if probe is None:
                        nat_a = nat_pool.tile([128, SLAB_B, CD], bf16, tag="nat_a")
                        nat_b = nat_pool.tile([68, SLAB_B, CD], bf16, tag="nat_b")
                        for j in range(SLAB_B):
                            rb = r0 + j * N_CTX
                            nc.sync.dma_start(nat_a[:, j, :],
                                              ctx_nat[rb:rb + 128, :])
                            nc.sync.dma_start(nat_b[:, j, :],
                                              ctx_nat[rb + 128:rb + N_CTX, :])

                    # ---- cp + tanh -> comb for both chunks of the slab ----
                    NH = SLAB_B // CHUNK_B  # 2
                    combs = [comb_pool.tile([128, NE, CHUNK_R], bf16,
                                            tag="comb", name=f"comb{_h}")
                             for _h in range(NH)]
                    pa_tiles = []
                    for e in range(NE):
                        es = slice(e * 128, (e + 1) * 128)
                        pcs = [pc_pool.tile([128, CHUNK_R], f32,
                                            tag="pcp", name=f"pc{_h}")
                               for _h in range(NH)]
                        n_mm = fp8_pairs + n16
                        mm_i = 0
                        for i in range(fp8_pairs):
                            ks = slice(2 * i, 2 * i + 2)
                            if swi:
                                w_ap = wc8_sb[:, i, e * 256:(e + 1) * 256]
                            else:
                                w_ap = wc8_sb[:, ks, es]
                            for h in range(NH):
                                rc = h * CHUNK_R
                                nc.tensor.matmul(
                                    pcs[h][:], w_ap,
                                    xt8[:, ks, rc:rc + CHUNK_R],
                                    start=(mm_i == 0), stop=(mm_i == n_mm - 1),
                                    perf_mode=DR,
                                )
                            mm_i += 1
                        for j in range(n16):
                            for h in range(NH):
                                rc = h * CHUNK_R
                                nc.tensor.matmul(
                                    pcs[h][:], wc16_sb[:, j, es],
                                    xt16[:, j, rc:rc + CHUNK_R],
                                    start=(mm_i == 0), stop=(mm_i == n_mm - 1),
                                )
                            mm_i += 1
                        if probe == "mm_cycle":
                            continue
                        for h in range(NH):
                            b0 = s * SLAB_B + h * CHUNK_B
                            for j in range(CHUNK_B):
                                nc.scalar.activation(
                                    combs[h][:, e, j * N_CTX:(j + 1) * N_CTX],
                                    pcs[h][:, j * N_CTX:(j + 1) * N_CTX],
                                    Act.Tanh,
                                    bias=qpb_sb[:, e, b0 + j:b0 + j + 1],
                                    scale=1.0 / WC_SCALE,
                                )

                    if probe is not None:
                        continue

                    # ---- attn.T = WoT.T @ comb.T -> [G, 196] per batch ----
                    for h in range(NH):
                        pa = pa_pool.tile([128, N_CTX], f32, tag="pat")
                        for e in range(NE):
                            for jj in range(CHUNK_B):
                                nc.tensor.matmul(
                                    pa[32 * jj:32 * jj + G, :],
                                    wo_sb[:, e, :],
                                    combs[h][:, e,
                                             jj * N_CTX:(jj + 1) * N_CTX],
                                    start=(e == 0), stop=(e == NE - 1),
                                    tile_position=(0, 32 * jj),
                                    skip_group_check=True,
                                )
                        asb = sm_pool.tile([128, N_CTX], f32, tag="asb")
                        for jj in range(CHUNK_B):
                            nc.scalar.activation(
                                asb[32 * jj:32 * jj + G, :],
                                pa[32 * jj:32 * jj + G, :], Act.Copy)
                        pa_tiles.append(asb)

                    # ---- softmax for all 4 batches, stacked at partition
                    # bases 0/32/64/96 (32-aligned) of one [128, N] tile
                    wexs = sm_pool.tile([128, N_CTX], bf16, tag="wexs")
                    ssum = sm_pool.tile([128, 1], f32, tag="ssum")
                    for j in range(SLAB_B):
                        pa = pa_tiles[j // CHUNK_B]
                        jj = j % CHUNK_B
                        seg = pa[32 * jj:32 * jj + G, :]
                        nmx = sm_pool.tile([G, 1], f32, tag="nmx")
                        nc.vector.tensor_reduce(
                            nmx[:], seg, axis=mybir.AxisListType.X,
                            op=Alu.max, negate=True)
                        nc.scalar.activation(
                            wexs[32 * j:32 * j + G, :], seg, Act.Exp,
                            bias=nmx[:], accum_out=ssum[32 * j:32 * j + G, :])
                    rs = sm_pool.tile([128, 1], f32, tag="rs")
                    nc.vector.reciprocal(rs[:], ssum[:])
                    if out_scale != 1.0:
                        nc.vector.tensor_scalar_mul(
                            rs[:], rs[:], float(out_scale))
                    rss = [rs[32 * j:32 * j + G, :] for j in range(SLAB_B)]

                    # one stacked transpose pair for the whole slab
                    wla = wl_pool.tile([128, 128], bf16, tag="wla")
                    wlb = wl_pool.tile([68, 128], bf16, tag="wlb")
                    pta = pa_pool.tile([128, 128], bf16, tag="pat")
                    nc.tensor.transpose(pta[:], wexs[:, 0:128], ident[:])
                    nc.vector.tensor_copy(wla[:], pta[:])
                    ptb = pa_pool.tile([68, 128], bf16, tag="pat")
                    nc.tensor.transpose(ptb[:], wexs[:, 128:N_CTX], ident[:])
                    nc.vector.tensor_copy(wlb[:], ptb[:])
                    wls = [(wla[:, 32 * j:32 * j + G],
                            wlb[:, 32 * j:32 * j + G])
                           for j in range(SLAB_B)]

                    # ---- glimpse: 4 batches via PE column tiling ----
                    outb = outb_pool.tile([128, CD], f32, tag="outb")
                    for cc in range(CD // 512):
                        pg = pg_pool.tile([128, 512], f32, tag="pgl")
                        for j in range(SLAB_B):
                            nc.tensor.matmul(
                                pg[32 * j:32 * j + G, :], wls[j][0],
                                nat_a[:, j, cc * 512:(cc + 1) * 512],
                                start=True, stop=False,
                                tile_position=(0, 32 * j),
                                skip_group_check=True)
                        for j in range(SLAB_B):
                            nc.tensor.matmul(
                                pg[32 * j:32 * j + G, :], wls[j][1],
                                nat_b[:, j, cc * 512:(cc + 1) * 512],
                                start=False, stop=True,
                                tile_position=(0, 32 * j),
                                skip_group_check=True)
                        for j in range(SLAB_B):
                            dst = outb[32 * j:32 * j + G,
                                       cc * 512:(cc + 1) * 512]
                            if j % 2 == 0:
                                nc.vector.tensor_scalar_mul(
                                    dst, pg[32 * j:32 * j + G, :], rss[j])
                            else:
                                nc.scalar.activation(
                                    dst, pg[32 * j:32 * j + G, :],
                                    Act.Identity, bias=0.0, scale=rss[j])

                    for j in range(SLAB_B):
                        nc.gpsimd.dma_start(
                            out[s * SLAB_B + j, :].rearrange(
                                "(g c) -> g c", g=G),
                            outb[32 * j:32 * j + G, :])

            with tc.tile_pool(name="wq", bufs=2) as wq_pool:
                for _rep in range(reps):
                    one_pass()

    nc.compile()
    return nc


_NC_CACHE = {}


def _get_nc(b_loc=B_LOC, fp8_pairs=6, swi=True):
    key = (b_loc, fp8_pairs, swi)
    if key not in _NC_CACHE:
        _NC_CACHE[key] = build_nc(b_loc, fp8_pairs=fp8_pairs, swi=swi)
    return _NC_CACHE[key]


def _swi_interleave(WcTs, fp8_pairs):
    """[Cd, E] pre-scaled Wc.T -> SwInterleave fp8 weight layout
    [fp8_pairs*128, NE*256]: per pair tile, free dim holds e-tiles of
    (A[:, ::-1], B[:, ::-1]) column-interleaved."""
    w8 = WcTs[:2 * fp8_pairs * 128].astype(FP8)
    out = np.empty((fp8_pairs, 128, NE, 256), dtype=FP8)
    for i in range(fp8_pairs):
        A = np.asarray(w8[(2 * i) * 128:(2 * i + 1) * 128]).reshape(128, NE, 128)
        B = np.asarray(
            w8[(2 * i + 1) * 128:(2 * i + 2) * 128]).reshape(128, NE, 128)
        out[i, :, :, 0::2] = A[:, :, ::-1]
        out[i, :, :, 1::2] = B[:, :, ::-1]
    return np.ascontiguousarray(out.reshape(fp8_pairs * 128, NE * 256))


def make_in_maps(context, query, Wq, bq, Wc, bc, Wo, bo, b_loc=B_LOC,
                 n_cores=N_CORES, fp8_pairs=6, swi=True):
    """Host-side prep: dtype conversion, transposes, quantization, sharding."""
    n8 = 2 * fp8_pairs
    n16 = NCC - n8
    c_split = n8 * 128

    context = np.asarray(context, dtype=np.float32)
    query = np.asarray(query)
    Wq = np.asarray(Wq)
    bq, bc_ = np.asarray(bq), np.asarray(bc)
    Wc = np.asarray(Wc, dtype=np.float32)
    Wo = np.asarray(Wo)

    ctx_bf = np.ascontiguousarray(context).astype(BF16)
    WqT = np.ascontiguousarray(Wq.T).astype(BF16)
    WoT = np.ascontiguousarray(np.asarray(Wo).T).astype(BF16)
    bqc = np.ascontiguousarray(
        (bq + bc_).astype(np.float32).reshape(E // 128, 128).T)

    WcTs = np.ascontiguousarray(Wc.T * WC_SCALE)  # [Cd, E], pre-scaled
    if n8:
        wc8T = (_swi_interleave(WcTs, fp8_pairs) if swi
                else WcTs[:c_split].astype(FP8))
    else:
        wc8T = None
    wc16T = WcTs[c_split:].astype(BF16) if n16 else None

    in_maps = []
    for i in range(n_cores):
        b0 = i * b_loc
        ctx_i = context[b0:b0 + b_loc].reshape(b_loc * N_CTX, CD)
        m = dict(
            ctxn=ctx_bf[b0:b0 + b_loc].reshape(b_loc * N_CTX, CD),
            qT=np.ascontiguousarray(query[b0:b0 + b_loc].T).astype(BF16),
            WqT=WqT, WoT=WoT, bqc=bqc,
        )
        if n8:
            m["ctx8T"] = np.ascontiguousarray(ctx_i[:, :c_split].T).astype(FP8)
            m["wc8T"] = wc8T
        if n16:
            m["ctx16T"] = np.ascontiguousarray(
                ctx_i[:, c_split:].T).astype(BF16)
            m["wc16T"] = wc16T
        in_maps.append(m)
    return in_maps


def kernel(context, query, Wq, bq, Wc, bc, Wo, bo, fp8_pairs=6, swi=True):
    from concourse.bass_utils import run_bass_kernel_spmd

    assert context.shape == (B_FULL, N_CTX, CD)
    nc = _get_nc(fp8_pairs=fp8_pairs, swi=swi)
    in_maps = make_in_maps(context, query, Wq, bq, Wc, bc, Wo, bo,
                           fp8_pairs=fp8_pairs, swi=swi)
    res = run_bass_kernel_spmd(nc, in_maps, core_ids=list(range(N_CORES)))
    return np.concatenate([res.results[i]["out"] for i in range(N_CORES)],
                          axis=0)


# revision 3
# speedup vs baseline: 1.6570x; 1.0090x over previous
"""Hybrid fp8-DoubleRow / bf16 Trainium2 kernel for the additive-attention
glimpse module.

Math (per batch b):
    qp  = query @ Wq.T + bq                       # [E]
    cp  = context @ Wc.T + bc                     # [N, E]
    comb = tanh(qp + cp)                          # [N, E]
    attn = comb @ Wo.T (+ bo, softmax-invariant)  # [N, G]
    w    = softmax(attn, axis=N)                  # [N, G]
    out  = (w.T @ context).reshape(G*Cd)          # [G*Cd]

Shapes: B=256, N=196, Cd=2048, Qd=E=1024, G=8.  Data-parallel over B on 8
cores (32 batches each).

The dominant matmul cp.T = Wc @ ctx.T (~26 GFLOP/core) runs with the first
`2*fp8_pairs` of the 16 contraction k-tiles as fp8e4m3 DoubleRow pairs
(2 k-tiles per PE pass, 2x throughput) and the rest in bf16.  Both operand
halves are pre-transposed and pre-quantized on the host; Wc is pre-scaled
by 32 (fp8 subnormal avoidance) and the tanh activation applies the 1/32
compensation via its scale parameter.
"""

import numpy as np
import ml_dtypes

BF16 = ml_dtypes.bfloat16
FP8 = ml_dtypes.float8_e4m3  # IEEE-style e4m3, max 240 == TRN FP8_EXP4

B_FULL = 256
N_CTX = 196
CD = 2048
QD = 1024
E = 1024
G = 8
N_CORES = 8
B_LOC = B_FULL // N_CORES  # 32

SLAB_B = 4                  # batches per slab
CHUNK_B = 2                 # batches per psum chunk (392 <= 512 psum bank)
CHUNK_R = CHUNK_B * N_CTX   # 392
R_SLAB = SLAB_B * N_CTX     # 784

WC_SCALE = 32.0             # host pre-scale on Wc (both halves)

NE = E // 128    # 8 e-tiles
NCC = CD // 128  # 16 c-tiles
NQ = QD // 128   # 8 q-tiles


def build_nc(b_loc=B_LOC, fp8_pairs=6, reps=1, probe=None, swi=True):
    import concourse.mybir as mybir
    import concourse.tile as tile
    from concourse import bacc
    from concourse.masks import make_identity

    f32 = mybir.dt.float32
    bf16 = mybir.dt.bfloat16
    fp8 = mybir.dt.float8e4
    Act = mybir.ActivationFunctionType
    Alu = mybir.AluOpType
    DR = (mybir.MatmulPerfMode.DoubleRowSwInterleave if swi
          else mybir.MatmulPerfMode.DoubleRow)

    n8 = 2 * fp8_pairs       # fp8 k-tiles
    n16 = NCC - n8           # bf16 k-tiles
    assert 0 <= n8 <= NCC

    assert b_loc % SLAB_B == 0
    n_slab = b_loc // SLAB_B
    R = b_loc * N_CTX

    nc = bacc.Bacc("TRN2", target_bir_lowering=False, debug=False,
                   num_devices=N_CORES)

    ctx_nat = nc.dram_tensor("ctxn", [R, CD], bf16, kind="ExternalInput").ap()
    if n8:
        ctx8T = nc.dram_tensor("ctx8T", [n8 * 128, R], fp8,
                               kind="ExternalInput").ap()
        if swi:
            wc8T = nc.dram_tensor("wc8T", [fp8_pairs * 128, NE * 256], fp8,
                                  kind="ExternalInput").ap()
        else:
            wc8T = nc.dram_tensor("wc8T", [n8 * 128, E], fp8,
                                  kind="ExternalInput").ap()
    if n16:
        ctx16T = nc.dram_tensor("ctx16T", [n16 * 128, R], bf16,
                                kind="ExternalInput").ap()
        wc16T = nc.dram_tensor("wc16T", [n16 * 128, E], bf16,
                               kind="ExternalInput").ap()
    qT = nc.dram_tensor("qT", [QD, b_loc], bf16, kind="ExternalInput").ap()
    WqT = nc.dram_tensor("WqT", [QD, E], bf16, kind="ExternalInput").ap()
    WoT = nc.dram_tensor("WoT", [E, G], bf16, kind="ExternalInput").ap()
    bqc = nc.dram_tensor("bqc", [128, E // 128], f32, kind="ExternalInput").ap()
    out = nc.dram_tensor("out", [b_loc, G * CD], f32, kind="ExternalOutput").ap()

    with tile.TileContext(nc) as tc:
        with (
            tc.tile_pool(name="const", bufs=1) as const_pool,
            tc.tile_pool(name="xt", bufs=2) as xt_pool,
            tc.tile_pool(name="nat", bufs=2) as nat_pool,
            tc.tile_pool(name="comb", bufs=4) as comb_pool,
            tc.tile_pool(name="sm", bufs=8) as sm_pool,
            tc.tile_pool(name="wl", bufs=4) as wl_pool,
            tc.tile_pool(name="outb", bufs=2) as outb_pool,
            tc.tile_pool(name="pcp", bufs=4, space="PSUM") as pc_pool,
            tc.tile_pool(name="pat", bufs=2, space="PSUM") as pa_pool,
            tc.tile_pool(name="pgl", bufs=2, space="PSUM") as pg_pool,
        ):
            # ---- persistent constants ----
            if n8:
                if swi:
                    wc8_sb = const_pool.tile([128, fp8_pairs, NE * 256], fp8)
                    nc.sync.dma_start(
                        wc8_sb[:], wc8T.rearrange("(k p) x -> p k x", p=128))
                else:
                    wc8_sb = const_pool.tile([128, n8, E], fp8)
                    nc.sync.dma_start(
                        wc8_sb[:], wc8T.rearrange("(k p) e -> p k e", p=128))
            if n16:
                wc16_sb = const_pool.tile([128, n16, E], bf16)
                nc.sync.dma_start(
                    wc16_sb[:], wc16T.rearrange("(k p) e -> p k e", p=128))
            wo_sb = const_pool.tile([128, NE, G], bf16)
            nc.sync.dma_start(wo_sb[:], WoT.rearrange("(k p) g -> p k g", p=128))
            bqc_sb = const_pool.tile([128, NE], f32)
            nc.sync.dma_start(bqc_sb[:], bqc[:])
            ident = const_pool.tile([128, 128], bf16)
            make_identity(nc, ident[:])

            qpb_sb = const_pool.tile([128, NE, b_loc], f32, tag="qpb")

            def one_pass(out_scale=1.0):
                # ---- qp = Wq @ query.T (+bq+bc), kept as [e, b] ----
                qt_sb = wq_pool.tile([128, NQ, b_loc], bf16, tag="qt")
                nc.sync.dma_start(qt_sb[:], qT.rearrange("(k p) b -> p k b", p=128))
                for e in range(NE):
                    wq_sb = wq_pool.tile([128, NQ, 128], bf16, tag="wqe")
                    nc.sync.dma_start(
                        wq_sb[:],
                        WqT[:, e * 128:(e + 1) * 128].rearrange(
                            "(k p) m -> p k m", p=128))
                    pq = pc_pool.tile([128, b_loc], f32, tag="pcp")
                    for k in range(NQ):
                        nc.tensor.matmul(
                            pq[:], wq_sb[:, k, :],
                            qt_sb[:, k, :], start=(k == 0), stop=(k == NQ - 1),
                        )
                    nc.vector.tensor_scalar_add(
                        qpb_sb[:, e, :], pq[:], bqc_sb[:, e:e + 1])

                # ---- main loop over 4-batch slabs ----
                for s in range(n_slab):
                    r0 = s * R_SLAB
                    if n8:
                        xt8 = xt_pool.tile([128, n8, R_SLAB], fp8, tag="xt8")
                        nc.sync.dma_start(
                            xt8[:],
                            ctx8T.rearrange("(k p) r -> p k r",
                                            p=128)[:, :, r0:r0 + R_SLAB])
                    if n16:
                        xt16 = xt_pool.tile([128, n16, R_SLAB], bf16, tag="xt16")
                        nc.sync.dma_start(
                            xt16[:],
                            ctx16T.rearrange("(k p) r -> p k r",
                                             p=128)[:, :, r0:r0 + R_SLAB])

                    if probe is None:
                        nat_a = nat_pool.tile([128, SLAB_B, CD], bf16, tag="nat_a")
                        nat_b = nat_pool.tile([68, SLAB_B, CD], bf16, tag="nat_b")
                        for j in range(SLAB_B):
                            rb = r0 + j * N_CTX
                            nc.sync.dma_start(nat_a[:, j, :],
                                              ctx_nat[rb:rb + 128, :])
                            nc.sync.dma_start(nat_b[:, j, :],
                                              ctx_nat[rb + 128:rb + N_CTX, :])

                    # ---- cp + tanh -> comb for both chunks of the slab ----
                    NH = SLAB_B // CHUNK_B  # 2
                    combs = [comb_pool.tile([128, NE, CHUNK_R], bf16,
                                            tag="comb", name=f"comb{_h}")
                             for _h in range(NH)]
                    pa_tiles = []
                    for e in range(NE):
                        es = slice(e * 128, (e + 1) * 128)
                        pcs = [pc_pool.tile([128, CHUNK_R], f32,
                                            tag="pcp", name=f"pc{_h}")
                               for _h in range(NH)]
                        n_mm = fp8_pairs + n16
                        mm_i = 0
                        for i in range(fp8_pairs):
                            ks = slice(2 * i, 2 * i + 2)
                            if swi:
                                w_ap = wc8_sb[:, i, e * 256:(e + 1) * 256]
                            else:
                                w_ap = wc8_sb[:, ks, es]
                            for h in range(NH):
                                rc = h * CHUNK_R
                                nc.tensor.matmul(
                                    pcs[h][:], w_ap,
                                    xt8[:, ks, rc:rc + CHUNK_R],
                                    start=(mm_i == 0), stop=(mm_i == n_mm - 1),
                                    perf_mode=DR,
                                )
                            mm_i += 1
                        for j in range(n16):
                            for h in range(NH):
                                rc = h * CHUNK_R
                                nc.tensor.matmul(
                                    pcs[h][:], wc16_sb[:, j, es],
                                    xt16[:, j, rc:rc + CHUNK_R],
                                    start=(mm_i == 0), stop=(mm_i == n_mm - 1),
                                )
                            mm_i += 1
                        if probe == "mm_cycle":
                            continue
                        for h in range(NH):
                            b0 = s * SLAB_B + h * CHUNK_B
                            for j in range(CHUNK_B):
                                nc.scalar.activation(
                                    combs[h][:, e, j * N_CTX:(j + 1) * N_CTX],
                                    pcs[h][:, j * N_CTX:(j + 1) * N_CTX],
                                    Act.Tanh,
                                    bias=qpb_sb[:, e, b0 + j:b0 + j + 1],
                                    scale=1.0 / WC_SCALE,
                                )

                    if probe is not None:
                        continue

                    # ---- attn.T = WoT.T @ comb.T -> [G, 196] per batch ----
                    for h in range(NH):
                        pa = pa_pool.tile([128, N_CTX], f32, tag="pat")
                        for e in range(NE):
                            for jj in range(CHUNK_B):
                                nc.tensor.matmul(
                                    pa[32 * jj:32 * jj + G, :],
                                    wo_sb[:, e, :],
                                    combs[h][:, e,
                                             jj * N_CTX:(jj + 1) * N_CTX],
                                    start=(e == 0), stop=(e == NE - 1),
                                    tile_position=(0, 32 * jj),
                                    skip_group_check=True,
                                )
                        asb = sm_pool.tile([128, N_CTX], f32, tag="asb")
                        for jj in range(CHUNK_B):
                            nc.scalar.activation(
                                asb[32 * jj:32 * jj + G, :],
                                pa[32 * jj:32 * jj + G, :], Act.Copy)
                        pa_tiles.append(asb)

                    # ---- softmax for all 4 batches, stacked at partition
                    # bases 0/32/64/96 (32-aligned) of one [128, N] tile
                    wexs = sm_pool.tile([128, N_CTX], bf16, tag="wexs")
                    ssum = sm_pool.tile([128, 1], f32, tag="ssum")
                    for j in range(SLAB_B):
                        pa = pa_tiles[j // CHUNK_B]
                        jj = j % CHUNK_B
                        seg = pa[32 * jj:32 * jj + G, :]
                        nmx = sm_pool.tile([G, 1], f32, tag="nmx")
                        nc.vector.tensor_reduce(
                            nmx[:], seg, axis=mybir.AxisListType.X,
                            op=Alu.max, negate=True)
                        nc.scalar.activation(
                            wexs[32 * j:32 * j + G, :], seg, Act.Exp,
                            bias=nmx[:], accum_out=ssum[32 * j:32 * j + G, :])
                    rs = sm_pool.tile([128, 1], f32, tag="rs")
                    nc.vector.reciprocal(rs[:], ssum[:])
                    if out_scale != 1.0:
                        nc.vector.tensor_scalar_mul(
                            rs[:], rs[:], float(out_scale))
                    rss = [rs[32 * j:32 * j + G, :] for j in range(SLAB_B)]

                    # tail (transposes+glimpse+out) for the PREVIOUS slab is
                    # emitted after this slab's cp/attn so the PE never waits
                    # on the softmax ACT/DVE chain.
                    if pending[0] is not None:
                        emit_tail(*pending[0])
                    pending[0] = (s, wexs, rss, nat_a, nat_b)

                if probe is None and pending[0] is not None:
                    emit_tail(*pending[0])
                    pending[0] = None

            def emit_tail(s, wexs, rss, nat_a, nat_b):
                # one stacked transpose pair for the whole slab
                wla = wl_pool.tile([128, 128], bf16, tag="wla")
                wlb = wl_pool.tile([68, 128], bf16, tag="wlb")
                pta = pg_pool.tile([128, 128], bf16, tag="pgl")
                nc.tensor.transpose(pta[:], wexs[:, 0:128], ident[:])
                nc.vector.tensor_copy(wla[:], pta[:])
                ptb = pg_pool.tile([68, 128], bf16, tag="pgl")
                nc.tensor.transpose(ptb[:], wexs[:, 128:N_CTX], ident[:])
                nc.vector.tensor_copy(wlb[:], ptb[:])
                wls = [(wla[:, 32 * j:32 * j + G],
                        wlb[:, 32 * j:32 * j + G])
                       for j in range(SLAB_B)]

                # ---- glimpse: 4 batches via PE column tiling ----
                outb = outb_pool.tile([128, CD], f32, tag="outb")
                for cc in range(CD // 512):
                    pg = pg_pool.tile([128, 512], f32, tag="pgl")
                    for j in range(SLAB_B):
                        nc.tensor.matmul(
                            pg[32 * j:32 * j + G, :], wls[j][0],
                            nat_a[:, j, cc * 512:(cc + 1) * 512],
                            start=True, stop=False,
                            tile_position=(0, 32 * j),
                            skip_group_check=True)
                    for j in range(SLAB_B):
                        nc.tensor.matmul(
                            pg[32 * j:32 * j + G, :], wls[j][1],
                            nat_b[:, j, cc * 512:(cc + 1) * 512],
                            start=False, stop=True,
                            tile_position=(0, 32 * j),
                            skip_group_check=True)
                    for j in range(SLAB_B):
                        dst = outb[32 * j:32 * j + G,
                                   cc * 512:(cc + 1) * 512]
                        if j % 2 == 0:
                            nc.vector.tensor_scalar_mul(
                                dst, pg[32 * j:32 * j + G, :], rss[j])
                        else:
                            nc.scalar.activation(
                                dst, pg[32 * j:32 * j + G, :],
                                Act.Identity, bias=0.0, scale=rss[j])

                for j in range(SLAB_B):
                    nc.gpsimd.dma_start(
                        out[s * SLAB_B + j, :].rearrange(
                            "(g c) -> g c", g=G),
                        outb[32 * j:32 * j + G, :])

            with tc.tile_pool(name="wq", bufs=2) as wq_pool:
                pending = [None]
                for _rep in range(reps):
                    one_pass()

    nc.compile()
    return nc


_NC_CACHE = {}


def _get_nc(b_loc=B_LOC, fp8_pairs=6, swi=True):
    key = (b_loc, fp8_pairs, swi)
    if key not in _NC_CACHE:
        _NC_CACHE[key] = build_nc(b_loc, fp8_pairs=fp8_pairs, swi=swi)
    return _NC_CACHE[key]


def _swi_interleave(WcTs, fp8_pairs):
    """[Cd, E] pre-scaled Wc.T -> SwInterleave fp8 weight layout
    [fp8_pairs*128, NE*256]: per pair tile, free dim holds e-tiles of
    (A[:, ::-1], B[:, ::-1]) column-interleaved."""
    w8 = WcTs[:2 * fp8_pairs * 128].astype(FP8)
    out = np.empty((fp8_pairs, 128, NE, 256), dtype=FP8)
    for i in range(fp8_pairs):
        A = np.asarray(w8[(2 * i) * 128:(2 * i + 1) * 128]).reshape(128, NE, 128)
        B = np.asarray(
            w8[(2 * i + 1) * 128:(2 * i + 2) * 128]).reshape(128, NE, 128)
        out[i, :, :, 0::2] = A[:, :, ::-1]
        out[i, :, :, 1::2] = B[:, :, ::-1]
    return np.ascontiguousarray(out.reshape(fp8_pairs * 128, NE * 256))


def make_in_maps(context, query, Wq, bq, Wc, bc, Wo, bo, b_loc=B_LOC,
                 n_cores=N_CORES, fp8_pairs=6, swi=True):
    """Host-side prep: dtype conversion, transposes, quantization, sharding."""
    n8 = 2 * fp8_pairs
    n16 = NCC - n8
    c_split = n8 * 128

    context = np.asarray(context, dtype=np.float32)
    query = np.asarray(query)
    Wq = np.asarray(Wq)
    bq, bc_ = np.asarray(bq), np.asarray(bc)
    Wc = np.asarray(Wc, dtype=np.float32)
    Wo = np.asarray(Wo)

    ctx_bf = np.ascontiguousarray(context).astype(BF16)
    WqT = np.ascontiguousarray(Wq.T).astype(BF16)
    WoT = np.ascontiguousarray(np.asarray(Wo).T).astype(BF16)
    bqc = np.ascontiguousarray(
        (bq + bc_).astype(np.float32).reshape(E // 128, 128).T)

    WcTs = np.ascontiguousarray(Wc.T * WC_SCALE)  # [Cd, E], pre-scaled
    if n8:
        wc8T = (_swi_interleave(WcTs, fp8_pairs) if swi
                else WcTs[:c_split].astype(FP8))
    else:
        wc8T = None
    wc16T = WcTs[c_split:].astype(BF16) if n16 else None

    in_maps = []
    for i in range(n_cores):
        b0 = i * b_loc
        ctx_i = context[b0:b0 + b_loc].reshape(b_loc * N_CTX, CD)
        m = dict(
            ctxn=ctx_bf[b0:b0 + b_loc].reshape(b_loc * N_CTX, CD),
            qT=np.ascontiguousarray(query[b0:b0 + b_loc].T).astype(BF16),
            WqT=WqT, WoT=WoT, bqc=bqc,
        )
        if n8:
            m["ctx8T"] = np.ascontiguousarray(ctx_i[:, :c_split].T).astype(FP8)
            m["wc8T"] = wc8T
        if n16:
            m["ctx16T"] = np.ascontiguousarray(
                ctx_i[:, c_split:].T).astype(BF16)
            m["wc16T"] = wc16T
        in_maps.append(m)
    return in_maps


def kernel(context, query, Wq, bq, Wc, bc, Wo, bo, fp8_pairs=6, swi=True):
    from concourse.bass_utils import run_bass_kernel_spmd

    assert context.shape == (B_FULL, N_CTX, CD)
    nc = _get_nc(fp8_pairs=fp8_pairs, swi=swi)
    in_maps = make_in_maps(context, query, Wq, bq, Wc, bc, Wo, bo,
                           fp8_pairs=fp8_pairs, swi=swi)
    res = run_bass_kernel_spmd(nc, in_maps, core_ids=list(range(N_CORES)))
    return np.concatenate([res.results[i]["out"] for i in range(N_CORES)],
                          axis=0)


# revision 4
# speedup vs baseline: 1.6767x; 1.0119x over previous
"""Hybrid fp8-DoubleRow / bf16 Trainium2 kernel for the additive-attention
glimpse module.

Math (per batch b):
    qp  = query @ Wq.T + bq                       # [E]
    cp  = context @ Wc.T + bc                     # [N, E]
    comb = tanh(qp + cp)                          # [N, E]
    attn = comb @ Wo.T (+ bo, softmax-invariant)  # [N, G]
    w    = softmax(attn, axis=N)                  # [N, G]
    out  = (w.T @ context).reshape(G*Cd)          # [G*Cd]

Shapes: B=256, N=196, Cd=2048, Qd=E=1024, G=8.  Data-parallel over B on 8
cores (32 batches each).

The dominant matmul cp.T = Wc @ ctx.T (~26 GFLOP/core) runs with the first
`2*fp8_pairs` (default 12) of the 16 contraction k-tiles as fp8e4m3
DoubleRowSwInterleave pairs (2 k-tiles per PE pass, ~2x throughput; the
host pre-interleaves the fp8 weights so the weight load stays contiguous)
and the remaining 4 k-tiles in bf16 as the accuracy anchor (rel err
1.80e-2 < 2e-2 gate; pure fp8 would be 2.07e-2).  Both operand halves are
pre-transposed and pre-quantized on the host (fp8 cannot use the device
DMA transpose); Wc is pre-scaled by 32 (fp8 subnormal avoidance) and the
tanh activation applies the 1/32 compensation via its scale parameter.
Slab tails (softmax-weight transposes + glimpse + output DMA) are
software-pipelined one slab behind cp/attn so the PE never idles waiting
on the softmax ACT/DVE chain.
"""

import numpy as np
import ml_dtypes

BF16 = ml_dtypes.bfloat16
FP8 = ml_dtypes.float8_e4m3  # IEEE-style e4m3, max 240 == TRN FP8_EXP4

B_FULL = 256
N_CTX = 196
CD = 2048
QD = 1024
E = 1024
G = 8
N_CORES = 8
B_LOC = B_FULL // N_CORES  # 32

SLAB_B = 4                  # batches per slab
CHUNK_B = 2                 # batches per psum chunk (392 <= 512 psum bank)
CHUNK_R = CHUNK_B * N_CTX   # 392
R_SLAB = SLAB_B * N_CTX     # 784

WC_SCALE = 32.0             # host pre-scale on Wc (both halves)

NE = E // 128    # 8 e-tiles
NCC = CD // 128  # 16 c-tiles
NQ = QD // 128   # 8 q-tiles


def build_nc(b_loc=B_LOC, fp8_pairs=6, reps=1, probe=None, swi=True):
    import concourse.mybir as mybir
    import concourse.tile as tile
    from concourse import bacc
    from concourse.masks import make_identity

    f32 = mybir.dt.float32
    bf16 = mybir.dt.bfloat16
    fp8 = mybir.dt.float8e4
    Act = mybir.ActivationFunctionType
    Alu = mybir.AluOpType
    DR = (mybir.MatmulPerfMode.DoubleRowSwInterleave if swi
          else mybir.MatmulPerfMode.DoubleRow)

    n8 = 2 * fp8_pairs       # fp8 k-tiles
    n16 = NCC - n8           # bf16 k-tiles
    assert 0 <= n8 <= NCC

    assert b_loc % SLAB_B == 0
    n_slab = b_loc // SLAB_B
    R = b_loc * N_CTX

    nc = bacc.Bacc("TRN2", target_bir_lowering=False, debug=False,
                   num_devices=N_CORES)

    ctx_nat = nc.dram_tensor("ctxn", [R, CD], bf16, kind="ExternalInput").ap()
    if n8:
        ctx8T = nc.dram_tensor("ctx8T", [n8 * 128, R], fp8,
                               kind="ExternalInput").ap()
        if swi:
            wc8T = nc.dram_tensor("wc8T", [fp8_pairs * 128, NE * 256], fp8,
                                  kind="ExternalInput").ap()
        else:
            wc8T = nc.dram_tensor("wc8T", [n8 * 128, E], fp8,
                                  kind="ExternalInput").ap()
    if n16:
        ctx16T = nc.dram_tensor("ctx16T", [n16 * 128, R], bf16,
                                kind="ExternalInput").ap()
        wc16T = nc.dram_tensor("wc16T", [n16 * 128, E], bf16,
                               kind="ExternalInput").ap()
    qT = nc.dram_tensor("qT", [QD, b_loc], bf16, kind="ExternalInput").ap()
    WqT = nc.dram_tensor("WqT", [QD, E], bf16, kind="ExternalInput").ap()
    WoT = nc.dram_tensor("WoT", [E, G], bf16, kind="ExternalInput").ap()
    bqc = nc.dram_tensor("bqc", [128, E // 128], f32, kind="ExternalInput").ap()
    out = nc.dram_tensor("out", [b_loc, G * CD], f32, kind="ExternalOutput").ap()

    with tile.TileContext(nc) as tc:
        with (
            tc.tile_pool(name="const", bufs=1) as const_pool,
            tc.tile_pool(name="xt", bufs=2) as xt_pool,
            tc.tile_pool(name="nat", bufs=2) as nat_pool,
            tc.tile_pool(name="comb", bufs=4) as comb_pool,
            tc.tile_pool(name="sm", bufs=8) as sm_pool,
            tc.tile_pool(name="wl", bufs=4) as wl_pool,
            tc.tile_pool(name="outb", bufs=2) as outb_pool,
            tc.tile_pool(name="pcp", bufs=4, space="PSUM") as pc_pool,
            tc.tile_pool(name="pat", bufs=2, space="PSUM") as pa_pool,
            tc.tile_pool(name="pgl", bufs=2, space="PSUM") as pg_pool,
        ):
            # ---- persistent constants ----
            if n8:
                if swi:
                    wc8_sb = const_pool.tile([128, fp8_pairs, NE * 256], fp8)
                    nc.sync.dma_start(
                        wc8_sb[:], wc8T.rearrange("(k p) x -> p k x", p=128))
                else:
                    wc8_sb = const_pool.tile([128, n8, E], fp8)
                    nc.sync.dma_start(
                        wc8_sb[:], wc8T.rearrange("(k p) e -> p k e", p=128))
            if n16:
                wc16_sb = const_pool.tile([128, n16, E], bf16)
                nc.sync.dma_start(
                    wc16_sb[:], wc16T.rearrange("(k p) e -> p k e", p=128))
            wo_sb = const_pool.tile([128, NE, G], bf16)
            nc.sync.dma_start(wo_sb[:], WoT.rearrange("(k p) g -> p k g", p=128))
            bqc_sb = const_pool.tile([128, NE], f32)
            nc.sync.dma_start(bqc_sb[:], bqc[:])
            ident = const_pool.tile([128, 128], bf16)
            make_identity(nc, ident[:])

            qpb_sb = const_pool.tile([128, NE, b_loc], f32, tag="qpb")

            def one_pass(out_scale=1.0):
                # ---- qp = Wq @ query.T (+bq+bc), kept as [e, b] ----
                qt_sb = wq_pool.tile([128, NQ, b_loc], bf16, tag="qt")
                nc.sync.dma_start(qt_sb[:], qT.rearrange("(k p) b -> p k b", p=128))
                for e in range(NE):
                    wq_sb = wq_pool.tile([128, NQ, 128], bf16, tag="wqe")
                    nc.sync.dma_start(
                        wq_sb[:],
                        WqT[:, e * 128:(e + 1) * 128].rearrange(
                            "(k p) m -> p k m", p=128))
                    pq = pc_pool.tile([128, b_loc], f32, tag="pcp")
                    for k in range(NQ):
                        nc.tensor.matmul(
                            pq[:], wq_sb[:, k, :],
                            qt_sb[:, k, :], start=(k == 0), stop=(k == NQ - 1),
                        )
                    nc.vector.tensor_scalar_add(
                        qpb_sb[:, e, :], pq[:], bqc_sb[:, e:e + 1])

                # ---- main loop over 4-batch slabs ----
                for s in range(n_slab):
                    r0 = s * R_SLAB
                    if n8:
                        xt8 = xt_pool.tile([128, n8, R_SLAB], fp8, tag="xt8")
                        c8r = ctx8T.rearrange("(k p) r -> p k r", p=128)
                        for k in range(0, n8, 2):
                            nc.sync.dma_start(
                                xt8[:, k:k + 2, :],
                                c8r[:, k:k + 2, r0:r0 + R_SLAB])
                    if n16:
                        xt16 = xt_pool.tile([128, n16, R_SLAB], bf16, tag="xt16")
                        c16r = ctx16T.rearrange("(k p) r -> p k r", p=128)
                        for k in range(n16):
                            nc.sync.dma_start(
                                xt16[:, k, :],
                                c16r[:, k, r0:r0 + R_SLAB])

                    if probe is None:
                        nat_a = nat_pool.tile([128, SLAB_B, CD], bf16, tag="nat_a")
                        nat_b = nat_pool.tile([68, SLAB_B, CD], bf16, tag="nat_b")
                        for j in range(SLAB_B):
                            rb = r0 + j * N_CTX
                            nc.sync.dma_start(nat_a[:, j, :],
                                              ctx_nat[rb:rb + 128, :])
                            nc.sync.dma_start(nat_b[:, j, :],
                                              ctx_nat[rb + 128:rb + N_CTX, :])

                    # ---- cp + tanh -> comb for both chunks of the slab ----
                    NH = SLAB_B // CHUNK_B  # 2
                    combs = [comb_pool.tile([128, NE, CHUNK_R], bf16,
                                            tag="comb", name=f"comb{_h}")
                             for _h in range(NH)]
                    pa_tiles = []
                    for e in range(NE):
                        es = slice(e * 128, (e + 1) * 128)
                        pcs = [pc_pool.tile([128, CHUNK_R], f32,
                                            tag="pcp", name=f"pc{_h}")
                               for _h in range(NH)]
                        n_mm = fp8_pairs + n16
                        mm_i = 0
                        for i in range(fp8_pairs):
                            ks = slice(2 * i, 2 * i + 2)
                            if swi:
                                w_ap = wc8_sb[:, i, e * 256:(e + 1) * 256]
                            else:
                                w_ap = wc8_sb[:, ks, es]
                            for h in range(NH):
                                rc = h * CHUNK_R
                                nc.tensor.matmul(
                                    pcs[h][:], w_ap,
                                    xt8[:, ks, rc:rc + CHUNK_R],
                                    start=(mm_i == 0), stop=(mm_i == n_mm - 1),
                                    perf_mode=DR,
                                )
                            mm_i += 1
                        for j in range(n16):
                            for h in range(NH):
                                rc = h * CHUNK_R
                                nc.tensor.matmul(
                                    pcs[h][:], wc16_sb[:, j, es],
                                    xt16[:, j, rc:rc + CHUNK_R],
                                    start=(mm_i == 0), stop=(mm_i == n_mm - 1),
                                )
                            mm_i += 1
                        if probe == "mm_cycle":
                            continue
                        for h in range(NH):
                            b0 = s * SLAB_B + h * CHUNK_B
                            for j in range(CHUNK_B):
                                nc.scalar.activation(
                                    combs[h][:, e, j * N_CTX:(j + 1) * N_CTX],
                                    pcs[h][:, j * N_CTX:(j + 1) * N_CTX],
                                    Act.Tanh,
                                    bias=qpb_sb[:, e, b0 + j:b0 + j + 1],
                                    scale=1.0 / WC_SCALE,
                                )

                    if probe is not None:
                        continue

                    # ---- attn.T = WoT.T @ comb.T -> [G, 196] per batch ----
                    for h in range(NH):
                        pa = pa_pool.tile([128, N_CTX], f32, tag="pat")
                        for e in range(NE):
                            for jj in range(CHUNK_B):
                                nc.tensor.matmul(
                                    pa[32 * jj:32 * jj + G, :],
                                    wo_sb[:, e, :],
                                    combs[h][:, e,
                                             jj * N_CTX:(jj + 1) * N_CTX],
                                    start=(e == 0), stop=(e == NE - 1),
                                    tile_position=(0, 32 * jj),
                                    skip_group_check=True,
                                )
                        asb = sm_pool.tile([128, N_CTX], f32, tag="asb")
                        for jj in range(CHUNK_B):
                            nc.scalar.activation(
                                asb[32 * jj:32 * jj + G, :],
                                pa[32 * jj:32 * jj + G, :], Act.Copy)
                        pa_tiles.append(asb)

                    # ---- softmax for all 4 batches, stacked at partition
                    # bases 0/32/64/96 (32-aligned) of one [128, N] tile
                    wexs = sm_pool.tile([128, N_CTX], bf16, tag="wexs")
                    ssum = sm_pool.tile([128, 1], f32, tag="ssum")
                    for j in range(SLAB_B):
                        pa = pa_tiles[j // CHUNK_B]
                        jj = j % CHUNK_B
                        seg = pa[32 * jj:32 * jj + G, :]
                        nmx = sm_pool.tile([G, 1], f32, tag="nmx")
                        nc.vector.tensor_reduce(
                            nmx[:], seg, axis=mybir.AxisListType.X,
                            op=Alu.max, negate=True)
                        nc.scalar.activation(
                            wexs[32 * j:32 * j + G, :], seg, Act.Exp,
                            bias=nmx[:], accum_out=ssum[32 * j:32 * j + G, :])
                    rs = sm_pool.tile([128, 1], f32, tag="rs")
                    nc.vector.reciprocal(rs[:], ssum[:])
                    if out_scale != 1.0:
                        nc.vector.tensor_scalar_mul(
                            rs[:], rs[:], float(out_scale))
                    rss = [rs[32 * j:32 * j + G, :] for j in range(SLAB_B)]

                    # tail (transposes+glimpse+out) for the PREVIOUS slab is
                    # emitted after this slab's cp/attn so the PE never waits
                    # on the softmax ACT/DVE chain.
                    if pending[0] is not None:
                        emit_tail(*pending[0])
                    pending[0] = (s, wexs, rss, nat_a, nat_b)

                if probe is None and pending[0] is not None:
                    emit_tail(*pending[0])
                    pending[0] = None

            def emit_tail(s, wexs, rss, nat_a, nat_b):
                # one stacked transpose pair for the whole slab
                wla = wl_pool.tile([128, 128], bf16, tag="wla")
                wlb = wl_pool.tile([68, 128], bf16, tag="wlb")
                pta = pg_pool.tile([128, 128], bf16, tag="pgl")
                nc.tensor.transpose(pta[:], wexs[:, 0:128], ident[:])
                nc.vector.tensor_copy(wla[:], pta[:])
                ptb = pg_pool.tile([68, 128], bf16, tag="pgl")
                nc.tensor.transpose(ptb[:], wexs[:, 128:N_CTX], ident[:])
                nc.vector.tensor_copy(wlb[:], ptb[:])
                wls = [(wla[:, 32 * j:32 * j + G],
                        wlb[:, 32 * j:32 * j + G])
                       for j in range(SLAB_B)]

                # ---- glimpse: 4 batches via PE column tiling ----
                outb = outb_pool.tile([128, CD], f32, tag="outb")
                for cc in range(CD // 512):
                    pg = pg_pool.tile([128, 512], f32, tag="pgl")
                    for j in range(SLAB_B):
                        nc.tensor.matmul(
                            pg[32 * j:32 * j + G, :], wls[j][0],
                            nat_a[:, j, cc * 512:(cc + 1) * 512],
                            start=True, stop=False,
                            tile_position=(0, 32 * j),
                            skip_group_check=True)
                    for j in range(SLAB_B):
                        nc.tensor.matmul(
                            pg[32 * j:32 * j + G, :], wls[j][1],
                            nat_b[:, j, cc * 512:(cc + 1) * 512],
                            start=False, stop=True,
                            tile_position=(0, 32 * j),
                            skip_group_check=True)
                    for j in range(SLAB_B):
                        dst = outb[32 * j:32 * j + G,
                                   cc * 512:(cc + 1) * 512]
                        if j % 2 == 0:
                            nc.vector.tensor_scalar_mul(
                                dst, pg[32 * j:32 * j + G, :], rss[j])
                        else:
                            nc.scalar.activation(
                                dst, pg[32 * j:32 * j + G, :],
                                Act.Identity, bias=0.0, scale=rss[j])

                for j in range(SLAB_B):
                    nc.gpsimd.dma_start(
                        out[s * SLAB_B + j, :].rearrange(
                            "(g c) -> g c", g=G),
                        outb[32 * j:32 * j + G, :])

            with tc.tile_pool(name="wq", bufs=2) as wq_pool:
                pending = [None]
                for _rep in range(reps):
                    one_pass()

    nc.compile()
    return nc


_NC_CACHE = {}


def _get_nc(b_loc=B_LOC, fp8_pairs=6, swi=True):
    key = (b_loc, fp8_pairs, swi)
    if key not in _NC_CACHE:
        _NC_CACHE[key] = build_nc(b_loc, fp8_pairs=fp8_pairs, swi=swi)
    return _NC_CACHE[key]


def _swi_interleave(WcTs, fp8_pairs):
    """[Cd, E] pre-scaled Wc.T -> SwInterleave fp8 weight layout
    [fp8_pairs*128, NE*256]: per pair tile, free dim holds e-tiles of
    (A[:, ::-1], B[:, ::-1]) column-interleaved."""
    w8 = WcTs[:2 * fp8_pairs * 128].astype(FP8)
    out = np.empty((fp8_pairs, 128, NE, 256), dtype=FP8)
    for i in range(fp8_pairs):
        A = np.asarray(w8[(2 * i) * 128:(2 * i + 1) * 128]).reshape(128, NE, 128)
        B = np.asarray(
            w8[(2 * i + 1) * 128:(2 * i + 2) * 128]).reshape(128, NE, 128)
        out[i, :, :, 0::2] = A[:, :, ::-1]
        out[i, :, :, 1::2] = B[:, :, ::-1]
    return np.ascontiguousarray(out.reshape(fp8_pairs * 128, NE * 256))


def make_in_maps(context, query, Wq, bq, Wc, bc, Wo, bo, b_loc=B_LOC,
                 n_cores=N_CORES, fp8_pairs=6, swi=True):
    """Host-side prep: dtype conversion, transposes, quantization, sharding."""
    n8 = 2 * fp8_pairs
    n16 = NCC - n8
    c_split = n8 * 128

    context = np.asarray(context, dtype=np.float32)
    query = np.asarray(query)
    Wq = np.asarray(Wq)
    bq, bc_ = np.asarray(bq), np.asarray(bc)
    Wc = np.asarray(Wc, dtype=np.float32)
    Wo = np.asarray(Wo)

    ctx_bf = np.ascontiguousarray(context).astype(BF16)
    WqT = np.ascontiguousarray(Wq.T).astype(BF16)
    WoT = np.ascontiguousarray(np.asarray(Wo).T).astype(BF16)
    bqc = np.ascontiguousarray(
        (bq + bc_).astype(np.float32).reshape(E // 128, 128).T)

    WcTs = np.ascontiguousarray(Wc.T * WC_SCALE)  # [Cd, E], pre-scaled
    if n8:
        wc8T = (_swi_interleave(WcTs, fp8_pairs) if swi
                else WcTs[:c_split].astype(FP8))
    else:
        wc8T = None
    wc16T = WcTs[c_split:].astype(BF16) if n16 else None

    in_maps = []
    for i in range(n_cores):
        b0 = i * b_loc
        ctx_i = context[b0:b0 + b_loc].reshape(b_loc * N_CTX, CD)
        m = dict(
            ctxn=ctx_bf[b0:b0 + b_loc].reshape(b_loc * N_CTX, CD),
            qT=np.ascontiguousarray(query[b0:b0 + b_loc].T).astype(BF16),
            WqT=WqT, WoT=WoT, bqc=bqc,
        )
        if n8:
            m["ctx8T"] = np.ascontiguousarray(ctx_i[:, :c_split].T).astype(FP8)
            m["wc8T"] = wc8T
        if n16:
            m["ctx16T"] = np.ascontiguousarray(
                ctx_i[:, c_split:].T).astype(BF16)
            m["wc16T"] = wc16T
        in_maps.append(m)
    return in_maps


def kernel(context, query, Wq, bq, Wc, bc, Wo, bo, fp8_pairs=6, swi=True):
    from concourse.bass_utils import run_bass_kernel_spmd

    assert context.shape == (B_FULL, N_CTX, CD)
    nc = _get_nc(fp8_pairs=fp8_pairs, swi=swi)
    in_maps = make_in_maps(context, query, Wq, bq, Wc, bc, Wo, bo,
                           fp8_pairs=fp8_pairs, swi=swi)
    res = run_bass_kernel_spmd(nc, in_maps, core_ids=list(range(N_CORES)))
    return np.concatenate([res.results[i]["out"] for i in range(N_CORES)],
                          axis=0)


# revision 5
# speedup vs baseline: 1.6929x; 1.0097x over previous
"""Hybrid fp8-DoubleRow / bf16 Trainium2 kernel for the additive-attention
glimpse module.

Math (per batch b):
    qp  = query @ Wq.T + bq                       # [E]
    cp  = context @ Wc.T + bc                     # [N, E]
    comb = tanh(qp + cp)                          # [N, E]
    attn = comb @ Wo.T (+ bo, softmax-invariant)  # [N, G]
    w    = softmax(attn, axis=N)                  # [N, G]
    out  = (w.T @ context).reshape(G*Cd)          # [G*Cd]

Shapes: B=256, N=196, Cd=2048, Qd=E=1024, G=8.  Data-parallel over B on 8
cores (32 batches each).

The dominant matmul cp.T = Wc @ ctx.T (~26 GFLOP/core) runs with the first
`2*fp8_pairs` (default 12) of the 16 contraction k-tiles as fp8e4m3
DoubleRowSwInterleave pairs (2 k-tiles per PE pass, ~2x throughput; the
host pre-interleaves the fp8 weights so the weight load stays contiguous)
and the remaining 4 k-tiles in bf16 as the accuracy anchor (rel err
1.80e-2 < 2e-2 gate; pure fp8 would be 2.07e-2).  Both operand halves are
pre-transposed and pre-quantized on the host (fp8 cannot use the device
DMA transpose); Wc is pre-scaled by 32 (fp8 subnormal avoidance) and the
tanh activation applies the 1/32 compensation via its scale parameter.
attn matmuls for all 4 batches of a slab pack into the 4 PE column
groups of one PSUM tile; softmax reads PSUM directly and the output
scaling is one full-partition DVE op per 512-column block.  Slab tails
(softmax-weight transposes + glimpse + output DMA) are software-pipelined
one slab behind cp/attn so the PE never idles on the softmax chain.
"""

import numpy as np
import ml_dtypes

BF16 = ml_dtypes.bfloat16
FP8 = ml_dtypes.float8_e4m3  # IEEE-style e4m3, max 240 == TRN FP8_EXP4

B_FULL = 256
N_CTX = 196
CD = 2048
QD = 1024
E = 1024
G = 8
N_CORES = 8
B_LOC = B_FULL // N_CORES  # 32

SLAB_B = 4                  # batches per slab
CHUNK_B = 2                 # batches per psum chunk (392 <= 512 psum bank)
CHUNK_R = CHUNK_B * N_CTX   # 392
R_SLAB = SLAB_B * N_CTX     # 784

WC_SCALE = 32.0             # host pre-scale on Wc (both halves)

NE = E // 128    # 8 e-tiles
NCC = CD // 128  # 16 c-tiles
NQ = QD // 128   # 8 q-tiles


def build_nc(b_loc=B_LOC, fp8_pairs=6, reps=1, probe=None, swi=True):
    import concourse.mybir as mybir
    import concourse.tile as tile
    from concourse import bacc
    from concourse.masks import make_identity

    f32 = mybir.dt.float32
    bf16 = mybir.dt.bfloat16
    fp8 = mybir.dt.float8e4
    Act = mybir.ActivationFunctionType
    Alu = mybir.AluOpType
    DR = (mybir.MatmulPerfMode.DoubleRowSwInterleave if swi
          else mybir.MatmulPerfMode.DoubleRow)

    n8 = 2 * fp8_pairs       # fp8 k-tiles
    n16 = NCC - n8           # bf16 k-tiles
    assert 0 <= n8 <= NCC

    assert b_loc % SLAB_B == 0
    n_slab = b_loc // SLAB_B
    R = b_loc * N_CTX

    nc = bacc.Bacc("TRN2", target_bir_lowering=False, debug=False,
                   num_devices=N_CORES)

    ctx_nat = nc.dram_tensor("ctxn", [R, CD], bf16, kind="ExternalInput").ap()
    if n8:
        ctx8T = nc.dram_tensor("ctx8T", [n8 * 128, R], fp8,
                               kind="ExternalInput").ap()
        if swi:
            wc8T = nc.dram_tensor("wc8T", [fp8_pairs * 128, NE * 256], fp8,
                                  kind="ExternalInput").ap()
        else:
            wc8T = nc.dram_tensor("wc8T", [n8 * 128, E], fp8,
                                  kind="ExternalInput").ap()
    if n16:
        ctx16T = nc.dram_tensor("ctx16T", [n16 * 128, R], bf16,
                                kind="ExternalInput").ap()
        wc16T = nc.dram_tensor("wc16T", [n16 * 128, E], bf16,
                               kind="ExternalInput").ap()
    qT = nc.dram_tensor("qT", [QD, b_loc], bf16, kind="ExternalInput").ap()
    WqT = nc.dram_tensor("WqT", [QD, E], bf16, kind="ExternalInput").ap()
    WoT = nc.dram_tensor("WoT", [E, G], bf16, kind="ExternalInput").ap()
    bqc = nc.dram_tensor("bqc", [128, E // 128], f32, kind="ExternalInput").ap()
    out = nc.dram_tensor("out", [b_loc, G * CD], f32, kind="ExternalOutput").ap()

    with tile.TileContext(nc) as tc:
        with (
            tc.tile_pool(name="const", bufs=1) as const_pool,
            tc.tile_pool(name="xt", bufs=2) as xt_pool,
            tc.tile_pool(name="nat", bufs=2) as nat_pool,
            tc.tile_pool(name="comb", bufs=4) as comb_pool,
            tc.tile_pool(name="sm", bufs=8) as sm_pool,
            tc.tile_pool(name="wl", bufs=4) as wl_pool,
            tc.tile_pool(name="outb", bufs=2) as outb_pool,
            tc.tile_pool(name="pcp", bufs=4, space="PSUM") as pc_pool,
            tc.tile_pool(name="pat", bufs=2, space="PSUM") as pa_pool,
            tc.tile_pool(name="pgl", bufs=2, space="PSUM") as pg_pool,
        ):
            # ---- persistent constants ----
            if n8:
                if swi:
                    wc8_sb = const_pool.tile([128, fp8_pairs, NE * 256], fp8)
                    nc.sync.dma_start(
                        wc8_sb[:], wc8T.rearrange("(k p) x -> p k x", p=128))
                else:
                    wc8_sb = const_pool.tile([128, n8, E], fp8)
                    nc.sync.dma_start(
                        wc8_sb[:], wc8T.rearrange("(k p) e -> p k e", p=128))
            if n16:
                wc16_sb = const_pool.tile([128, n16, E], bf16)
                nc.sync.dma_start(
                    wc16_sb[:], wc16T.rearrange("(k p) e -> p k e", p=128))
            wo_sb = const_pool.tile([128, NE, G], bf16)
            nc.sync.dma_start(wo_sb[:], WoT.rearrange("(k p) g -> p k g", p=128))
            bqc_sb = const_pool.tile([128, NE], f32)
            nc.sync.dma_start(bqc_sb[:], bqc[:])
            ident = const_pool.tile([128, 128], bf16)
            make_identity(nc, ident[:])

            qpb_sb = const_pool.tile([128, NE, b_loc], f32, tag="qpb")

            def one_pass(out_scale=1.0):
                # ---- qp = Wq @ query.T (+bq+bc), kept as [e, b] ----
                qt_sb = wq_pool.tile([128, NQ, b_loc], bf16, tag="qt")
                nc.sync.dma_start(qt_sb[:], qT.rearrange("(k p) b -> p k b", p=128))
                for e in range(NE):
                    wq_sb = wq_pool.tile([128, NQ, 128], bf16, tag="wqe")
                    nc.sync.dma_start(
                        wq_sb[:],
                        WqT[:, e * 128:(e + 1) * 128].rearrange(
                            "(k p) m -> p k m", p=128))
                    pq = pc_pool.tile([128, b_loc], f32, tag="pcp")
                    for k in range(NQ):
                        nc.tensor.matmul(
                            pq[:], wq_sb[:, k, :],
                            qt_sb[:, k, :], start=(k == 0), stop=(k == NQ - 1),
                        )
                    nc.vector.tensor_scalar_add(
                        qpb_sb[:, e, :], pq[:], bqc_sb[:, e:e + 1])

                # ---- main loop over 4-batch slabs ----
                for s in range(n_slab):
                    r0 = s * R_SLAB
                    if n8:
                        xt8 = xt_pool.tile([128, n8, R_SLAB], fp8, tag="xt8")
                        c8r = ctx8T.rearrange("(k p) r -> p k r", p=128)
                        for k in range(0, n8, 2):
                            nc.sync.dma_start(
                                xt8[:, k:k + 2, :],
                                c8r[:, k:k + 2, r0:r0 + R_SLAB])
                    if n16:
                        xt16 = xt_pool.tile([128, n16, R_SLAB], bf16, tag="xt16")
                        c16r = ctx16T.rearrange("(k p) r -> p k r", p=128)
                        for k in range(n16):
                            nc.sync.dma_start(
                                xt16[:, k, :],
                                c16r[:, k, r0:r0 + R_SLAB])

                    if probe is None:
                        nat_a = nat_pool.tile([128, SLAB_B, CD], bf16, tag="nat_a")
                        nat_b = nat_pool.tile([68, SLAB_B, CD], bf16, tag="nat_b")
                        for j in range(SLAB_B):
                            rb = r0 + j * N_CTX
                            nc.sync.dma_start(nat_a[:, j, :],
                                              ctx_nat[rb:rb + 128, :])
                            nc.sync.dma_start(nat_b[:, j, :],
                                              ctx_nat[rb + 128:rb + N_CTX, :])

                    # ---- cp + tanh -> comb for both chunks of the slab ----
                    NH = SLAB_B // CHUNK_B  # 2
                    combs = [comb_pool.tile([128, NE, CHUNK_R], bf16,
                                            tag="comb", name=f"comb{_h}")
                             for _h in range(NH)]
                    for e in range(NE):
                        es = slice(e * 128, (e + 1) * 128)
                        pcs = [pc_pool.tile([128, CHUNK_R], f32,
                                            tag="pcp", name=f"pc{_h}")
                               for _h in range(NH)]
                        n_mm = fp8_pairs + n16
                        mm_i = 0
                        for i in range(fp8_pairs):
                            ks = slice(2 * i, 2 * i + 2)
                            if swi:
                                w_ap = wc8_sb[:, i, e * 256:(e + 1) * 256]
                            else:
                                w_ap = wc8_sb[:, ks, es]
                            for h in range(NH):
                                rc = h * CHUNK_R
                                nc.tensor.matmul(
                                    pcs[h][:], w_ap,
                                    xt8[:, ks, rc:rc + CHUNK_R],
                                    start=(mm_i == 0), stop=(mm_i == n_mm - 1),
                                    perf_mode=DR,
                                )
                            mm_i += 1
                        for j in range(n16):
                            for h in range(NH):
                                rc = h * CHUNK_R
                                nc.tensor.matmul(
                                    pcs[h][:], wc16_sb[:, j, es],
                                    xt16[:, j, rc:rc + CHUNK_R],
                                    start=(mm_i == 0), stop=(mm_i == n_mm - 1),
                                )
                            mm_i += 1
                        if probe == "mm_cycle":
                            continue
                        for h in range(NH):
                            b0 = s * SLAB_B + h * CHUNK_B
                            for j in range(CHUNK_B):
                                nc.scalar.activation(
                                    combs[h][:, e, j * N_CTX:(j + 1) * N_CTX],
                                    pcs[h][:, j * N_CTX:(j + 1) * N_CTX],
                                    Act.Tanh,
                                    bias=qpb_sb[:, e, b0 + j:b0 + j + 1],
                                    scale=1.0 / WC_SCALE,
                                )

                    if probe is not None:
                        continue

                    # ---- attn.T = WoT.T @ comb.T: all 4 batches packed into
                    # the 4 PE column groups of ONE [128, 196] psum tile ----
                    pa = pa_pool.tile([128, N_CTX], f32, tag="pat")
                    for e in range(NE):
                        for j4 in range(SLAB_B):
                            h, jj = divmod(j4, CHUNK_B)
                            nc.tensor.matmul(
                                pa[32 * j4:32 * j4 + G, :],
                                wo_sb[:, e, :],
                                combs[h][:, e, jj * N_CTX:(jj + 1) * N_CTX],
                                start=(e == 0), stop=(e == NE - 1),
                                tile_position=(0, 32 * j4),
                                skip_group_check=True,
                            )

                    # ---- softmax for all 4 batches, stacked at partition
                    # bases 0/32/64/96 (32-aligned), straight from PSUM ----
                    wexs = sm_pool.tile([128, N_CTX], bf16, tag="wexs")
                    ssum = sm_pool.tile([128, 1], f32, tag="ssum")
                    for j in range(SLAB_B):
                        seg = pa[32 * j:32 * j + G, :]
                        nmx = sm_pool.tile([G, 1], f32, tag="nmx")
                        nc.vector.tensor_reduce(
                            nmx[:], seg, axis=mybir.AxisListType.X,
                            op=Alu.max, negate=True)
                        nc.scalar.activation(
                            wexs[32 * j:32 * j + G, :], seg, Act.Exp,
                            bias=nmx[:], accum_out=ssum[32 * j:32 * j + G, :])
                    rs = sm_pool.tile([128, 1], f32, tag="rs")
                    nc.vector.reciprocal(rs[:], ssum[:])
                    if out_scale != 1.0:
                        nc.vector.tensor_scalar_mul(
                            rs[:], rs[:], float(out_scale))
                    # tail (transposes+glimpse+out) for the PREVIOUS slab is
                    # emitted after this slab's cp/attn so the PE never waits
                    # on the softmax ACT/DVE chain.
                    if pending[0] is not None:
                        emit_tail(*pending[0])
                    pending[0] = (s, wexs, rs, nat_a, nat_b)

                if probe is None and pending[0] is not None:
                    emit_tail(*pending[0])
                    pending[0] = None

            def emit_tail(s, wexs, rs_full, nat_a, nat_b):
                # one stacked transpose pair for the whole slab
                wla = wl_pool.tile([128, 128], bf16, tag="wla")
                wlb = wl_pool.tile([68, 128], bf16, tag="wlb")
                pta = pg_pool.tile([128, 128], bf16, tag="pgl")
                nc.tensor.transpose(pta[:], wexs[:, 0:128], ident[:])
                nc.vector.tensor_copy(wla[:], pta[:])
                ptb = pg_pool.tile([68, 128], bf16, tag="pgl")
                nc.tensor.transpose(ptb[:], wexs[:, 128:N_CTX], ident[:])
                nc.vector.tensor_copy(wlb[:], ptb[:])
                wls = [(wla[:, 32 * j:32 * j + G],
                        wlb[:, 32 * j:32 * j + G])
                       for j in range(SLAB_B)]

                # ---- glimpse: 4 batches via PE column tiling ----
                outb = outb_pool.tile([128, CD], f32, tag="outb")
                for cc in range(CD // 512):
                    pg = pg_pool.tile([128, 512], f32, tag="pgl")
                    for j in range(SLAB_B):
                        nc.tensor.matmul(
                            pg[32 * j:32 * j + G, :], wls[j][0],
                            nat_a[:, j, cc * 512:(cc + 1) * 512],
                            start=True, stop=False,
                            tile_position=(0, 32 * j),
                            skip_group_check=True)
                    for j in range(SLAB_B):
                        nc.tensor.matmul(
                            pg[32 * j:32 * j + G, :], wls[j][1],
                            nat_b[:, j, cc * 512:(cc + 1) * 512],
                            start=False, stop=True,
                            tile_position=(0, 32 * j),
                            skip_group_check=True)
                    # one full-partition scale: garbage rows (outside the
                    # 32j..32j+G groups) are scaled by garbage but never read
                    nc.vector.tensor_scalar_mul(
                        outb[:, cc * 512:(cc + 1) * 512], pg[:], rs_full)

                for j in range(SLAB_B):
                    nc.gpsimd.dma_start(
                        out[s * SLAB_B + j, :].rearrange(
                            "(g c) -> g c", g=G),
                        outb[32 * j:32 * j + G, :])

            with tc.tile_pool(name="wq", bufs=2) as wq_pool:
                pending = [None]
                for _rep in range(reps):
                    one_pass()

    nc.compile()
    return nc


_NC_CACHE = {}


def _get_nc(b_loc=B_LOC, fp8_pairs=6, swi=True):
    key = (b_loc, fp8_pairs, swi)
    if key not in _NC_CACHE:
        _NC_CACHE[key] = build_nc(b_loc, fp8_pairs=fp8_pairs, swi=swi)
    return _NC_CACHE[key]


def _swi_interleave(WcTs, fp8_pairs):
    """[Cd, E] pre-scaled Wc.T -> SwInterleave fp8 weight layout
    [fp8_pairs*128, NE*256]: per pair tile, free dim holds e-tiles of
    (A[:, ::-1], B[:, ::-1]) column-interleaved."""
    w8 = WcTs[:2 * fp8_pairs * 128].astype(FP8)
    out = np.empty((fp8_pairs, 128, NE, 256), dtype=FP8)
    for i in range(fp8_pairs):
        A = np.asarray(w8[(2 * i) * 128:(2 * i + 1) * 128]).reshape(128, NE, 128)
        B = np.asarray(
            w8[(2 * i + 1) * 128:(2 * i + 2) * 128]).reshape(128, NE, 128)
        out[i, :, :, 0::2] = A[:, :, ::-1]
        out[i, :, :, 1::2] = B[:, :, ::-1]
    return np.ascontiguousarray(out.reshape(fp8_pairs * 128, NE * 256))


def make_in_maps(context, query, Wq, bq, Wc, bc, Wo, bo, b_loc=B_LOC,
                 n_cores=N_CORES, fp8_pairs=6, swi=True):
    """Host-side prep: dtype conversion, transposes, quantization, sharding."""
    n8 = 2 * fp8_pairs
    n16 = NCC - n8
    c_split = n8 * 128

    context = np.asarray(context, dtype=np.float32)
    query = np.asarray(query)
    Wq = np.asarray(Wq)
    bq, bc_ = np.asarray(bq), np.asarray(bc)
    Wc = np.asarray(Wc, dtype=np.float32)
    Wo = np.asarray(Wo)

    ctx_bf = np.ascontiguousarray(context).astype(BF16)
    WqT = np.ascontiguousarray(Wq.T).astype(BF16)
    WoT = np.ascontiguousarray(np.asarray(Wo).T).astype(BF16)
    bqc = np.ascontiguousarray(
        (bq + bc_).astype(np.float32).reshape(E // 128, 128).T)

    WcTs = np.ascontiguousarray(Wc.T * WC_SCALE)  # [Cd, E], pre-scaled
    if n8:
        wc8T = (_swi_interleave(WcTs, fp8_pairs) if swi
                else WcTs[:c_split].astype(FP8))
    else:
        wc8T = None
    wc16T = WcTs[c_split:].astype(BF16) if n16 else None

    in_maps = []
    for i in range(n_cores):
        b0 = i * b_loc
        ctx_i = context[b0:b0 + b_loc].reshape(b_loc * N_CTX, CD)
        m = dict(
            ctxn=ctx_bf[b0:b0 + b_loc].reshape(b_loc * N_CTX, CD),
            qT=np.ascontiguousarray(query[b0:b0 + b_loc].T).astype(BF16),
            WqT=WqT, WoT=WoT, bqc=bqc,
        )
        if n8:
            m["ctx8T"] = np.ascontiguousarray(ctx_i[:, :c_split].T).astype(FP8)
            m["wc8T"] = wc8T
        if n16:
            m["ctx16T"] = np.ascontiguousarray(
                ctx_i[:, c_split:].T).astype(BF16)
            m["wc16T"] = wc16T
        in_maps.append(m)
    return in_maps


def kernel(context, query, Wq, bq, Wc, bc, Wo, bo, fp8_pairs=6, swi=True):
    from concourse.bass_utils import run_bass_kernel_spmd

    assert context.shape == (B_FULL, N_CTX, CD)
    nc = _get_nc(fp8_pairs=fp8_pairs, swi=swi)
    in_maps = make_in_maps(context, query, Wq, bq, Wc, bc, Wo, bo,
                           fp8_pairs=fp8_pairs, swi=swi)
    res = run_bass_kernel_spmd(nc, in_maps, core_ids=list(range(N_CORES)))
    return np.concatenate([res.results[i]["out"] for i in range(N_CORES)],
                          axis=0)
